# revision 2
# baseline (speedup 1.0000x reference)
"""GLM-style dual-RoPE attention block on 8 trn2 NeuronCores.

Sharding: tensor-parallel over heads (16 heads -> 2 per core).
Per core: QKV projection for its heads (transposed layout), dual RoPE,
full S x S attention (streamed softmax over key tiles, no max subtraction
-- max |logit| ~60 so exp stays in fp32 range), unnormalized P@V, late
normalization, and a partial output projection.  Partials are summed on
host; qkv v-bias is folded into a host-side constant row (sum_k p_k = 1),
attn_out bias added on host.

v2 changes vs baseline:
 - softmax denominators no longer use per-tile ones-matmuls on the PE
   (they cost a full 512-column stream each).  Instead the exp tiles are
   accumulated across key tiles on the DVE (two bf16 ping chains), and a
   single GpSimd partition_all_reduce produces the broadcasted sums.
 - rope bias add is fused into the cos multiply via scalar_tensor_tensor;
   the rotate-half sign is folded into the host sin table so the rotate is
   4 plain ACT copies.
 - x / weight DMAs are batched 4 k-tiles per transfer; weight/table
   triggers moved to the ACT hwdge queue so the sync queue only carries
   x and output tiles.
 - phase 2 and phase 3 share one PSUM pool scope (no inter-phase barrier).

All device matmuls run in float32r (full PE rate at N>=512).  Attention
works on 1024-wide query chunks; the P@V matmuls are software-pipelined
two key-tiles behind the logit matmuls so the ACT exp latency never
stalls the PE.
"""

import ml_dtypes
import numpy as np

import concourse.bass as bass
from concourse import bacc, bass_isa
import concourse.mybir as mybir
import concourse.tile as tile
from concourse.bass_utils import run_bass_kernel_spmd
from concourse.masks import make_identity

F32 = mybir.dt.float32
F32R = mybir.dt.float32r
BF16 = mybir.dt.bfloat16
AF = mybir.ActivationFunctionType
ALU = mybir.AluOpType

S, D, H, HD = 2048, 2048, 16, 128
NCORES = 8
HPC = H // NCORES          # heads per core = 2
KT = D // 128              # 16 contraction tiles
KG = KT // 4               # 4 contraction groups (4 tiles each)
ST = S // 128              # 16 key 128-tiles
QC = S // 512              # 4 sequence 512-chunks

DT_IN = F32R
DT_ATT = F32R

_LAST_RESULTS = None
_BUILT = None


def _build():
    nc = bacc.Bacc("TRN2", target_bir_lowering=False, debug=False,
                   num_devices=NCORES)
    xT_d = nc.dram_tensor("xT", [D, S], DT_IN, kind="ExternalInput").ap()
    wqk_d = nc.dram_tensor("wqk", [D, 4 * 128], DT_IN, kind="ExternalInput").ap()
    bqk_d = nc.dram_tensor("bqk", [128, 4], F32, kind="ExternalInput").ap()
    rbqk_d = nc.dram_tensor("rbqk", [128, 4], F32, kind="ExternalInput").ap()
    wv_d = nc.dram_tensor("wv", [D, HPC * 128], DT_IN, kind="ExternalInput").ap()
    cos_d = nc.dram_tensor("cos", [128, S], F32, kind="ExternalInput").ap()
    sin_d = nc.dram_tensor("sin", [128, S], F32, kind="ExternalInput").ap()
    wo_d = nc.dram_tensor("wo", [HPC * 128, D], BF16, kind="ExternalInput").ap()
    out_d = nc.dram_tensor("out", [S, D], BF16, kind="ExternalOutput").ap()

    with tile.TileContext(nc) as tc:
        with tc.tile_pool(name="res", bufs=1) as res:
            cos_sb = res.tile([128, S], F32, tag="cos")
            sin_sb = res.tile([128, S], F32, tag="sin")
            bqk_sb = res.tile([128, 4], F32, tag="bqk")
            rbqk_sb = res.tile([128, 4], F32, tag="rbqk")
            wo_sb = [res.tile([128, D], BF16, tag=f"wo{h}", name=f"wo{h}")
                     for h in range(HPC)]
            qkT = [[res.tile([128, 512], DT_ATT, tag=f"qkT{m}_{nq}",
                             name=f"qkT{m}_{nq}") for nq in range(QC)]
                   for m in range(4)]
            vT = [[res.tile([128, 512], DT_ATT, tag=f"vT{h}_{nq}",
                            name=f"vT{h}_{nq}") for nq in range(QC)]
                  for h in range(HPC)]
            vnat = [res.tile([128, 256], BF16, tag=f"vnat{st}",
                             name=f"vnat{st}") for st in range(ST)]

            # ---- phase 1: qkv^T = W^T @ x^T (streamed over s-quarters) ----
            with (
                tc.tile_pool(name="xs", bufs=4) as xs,
                tc.tile_pool(name="tmp", bufs=2) as tmp,
                tc.tile_pool(name="ps1", bufs=8, space="PSUM") as ps1,
            ):
                wqk_sb = [xs.tile([128, 4, 512], DT_IN, tag=f"wqk{g}",
                                  name=f"wqk{g}", bufs=1) for g in range(KG)]
                wv_sb = [xs.tile([128, 4, 256], DT_IN, tag=f"wvw{g}",
                                 name=f"wvw{g}", bufs=1) for g in range(KG)]
                for nq in range(QC):
                    ns = slice(nq * 512, (nq + 1) * 512)
                    psums = [ps1.tile([128, 512], F32, tag="ps",
                                      name=f"qkvps{nq}_{i}", bufs=8)
                             for i in range(6)]
                    for g in range(KG):
                        gs = slice(g * 512, (g + 1) * 512)
                        if nq == 0:
                            nc.scalar.dma_start(
                                wqk_sb[g][:],
                                wqk_d[gs, :].rearrange("(k p) c -> p k c", k=4))
                            nc.scalar.dma_start(
                                wv_sb[g][:],
                                wv_d[gs, :].rearrange("(k p) c -> p k c", k=4))
                        xt = xs.tile([128, 4, 512], DT_IN, tag="xt", bufs=3)
                        nc.sync.dma_start(
                            xt[:],
                            xT_d[gs, ns].rearrange("(k p) c -> p k c", k=4))
                        for ki in range(4):
                            k = g * 4 + ki
                            for m in range(4):
                                nc.tensor.matmul(
                                    psums[m][:],
                                    wqk_sb[g][:, ki, m * 128:(m + 1) * 128],
                                    xt[:, ki, :],
                                    start=(k == 0), stop=(k == KT - 1))
                            for h in range(HPC):
                                nc.tensor.matmul(
                                    psums[4 + h][:],
                                    wv_sb[g][:, ki, h * 128:(h + 1) * 128],
                                    xt[:, ki, :],
                                    start=(k == 0), stop=(k == KT - 1))
                        if nq == 0 and g == 0:
                            # tables needed from the first rope onward
                            nc.scalar.dma_start(cos_sb[:], cos_d[:, :])
                            nc.scalar.dma_start(sin_sb[:], sin_d[:, :])
                            nc.scalar.dma_start(bqk_sb[:], bqk_d[:, :])
                            nc.scalar.dma_start(rbqk_sb[:], rbqk_d[:, :])
                    # q/k eviction: t1 = (psum + b) * cos  (DVE, fused)
                    #               rot = rotate32(psum)   (4 ACT copies)
                    #               t2 = (rot + rot_b) * sin_signed (DVE)
                    #               qkT = t1 + t2          (DVE)
                    for m in range(4):
                        t1 = tmp.tile([128, 512], F32, tag="t1")
                        nc.vector.scalar_tensor_tensor(
                            t1[:], psums[m][:], bqk_sb[:, m:m + 1],
                            cos_sb[:, ns], op0=ALU.add, op1=ALU.mult)
                        rot = tmp.tile([128, 512], F32, tag="rot")
                        for b0 in (0, 64):
                            nc.scalar.copy(rot[b0:b0 + 32, :],
                                           psums[m][b0 + 32:b0 + 64, :])
                            nc.scalar.copy(rot[b0 + 32:b0 + 64, :],
                                           psums[m][b0:b0 + 32, :])
                        t2 = tmp.tile([128, 512], F32, tag="t2")
                        nc.vector.scalar_tensor_tensor(
                            t2[:], rot[:], rbqk_sb[:, m:m + 1],
                            sin_sb[:, ns], op0=ALU.add, op1=ALU.mult)
                        nc.vector.tensor_add(qkT[m][nq][:], t1[:], t2[:])
                    # v^T: plain eviction (bias folded on host)
                    for h in range(HPC):
                        nc.scalar.copy(vT[h][nq][:], psums[4 + h][:])

                # ---- phase 1b: transpose v^T -> v natural [s, vd] ----
                ident_f = res.tile([128, 128], F32, tag="ident_f")
                make_identity(nc, ident_f[:])
                ident = res.tile([128, 128], DT_ATT, tag="ident")
                nc.vector.tensor_copy(ident[:], ident_f[:])
                for h in range(HPC):
                    for st in range(ST):
                        tp = ps1.tile([128, 128], DT_ATT, tag="ps", bufs=8)
                        nc.tensor.transpose(
                            tp[:],
                            vT[h][st // 4][:, (st % 4) * 128:(st % 4 + 1) * 128],
                            ident[:])
                        nc.any.tensor_copy(vnat[st][:, h * 128:(h + 1) * 128],
                                           tp[:])

            # ---- phase 2: attention, 1024-wide query chunks ----
            # ---- phase 3: partial out-projection (same psum scope) ----
            wvn = [[res.tile([128, 512], BF16, tag=f"wvn{h}_{nq}",
                            name=f"wvn{h}_{nq}") for nq in range(QC)]
                   for h in range(HPC)]
            with (
                tc.tile_pool(name="ex", bufs=4) as exp_pool,
                tc.tile_pool(name="acc", bufs=1) as accp,
                tc.tile_pool(name="rp", bufs=2) as rp,
                tc.tile_pool(name="ob", bufs=2) as obp,
                tc.tile_pool(name="ps2", bufs=1, space="PSUM") as ps2,
            ):
                for h in range(HPC):
                    nc.scalar.dma_start(wo_sb[h][:],
                                        wo_d[h * 128:(h + 1) * 128, :])
                for qc in range(2):
                    for h in range(HPC):
                        qT_h = qkT[h]
                        kT_h = qkT[2 + h]
                        wv_ps = ps2.tile([128, 1024], F32, tag="wv", bufs=2,
                                         name=f"wvps{h}_{qc}")
                        acc_e = accp.tile([128, 1024], BF16, tag="acc_e",
                                          name=f"acce{h}_{qc}", bufs=2)
                        acc_o = accp.tile([128, 1024], BF16, tag="acc_o",
                                          name=f"acco{h}_{qc}", bufs=2)
                        exs = {}
                        for st in range(ST + 2):   # 2-deep software pipeline
                            if st < ST:
                                lg = ps2.tile([128, 1024], F32, tag="lg",
                                              bufs=2, name=f"lg{h}_{qc}_{st}")
                                kts = kT_h[st // 4][:, (st % 4) * 128:
                                                    (st % 4 + 1) * 128]
                                for half in range(2):
                                    nc.tensor.matmul(
                                        lg[:, half * 512:(half + 1) * 512],
                                        kts,
                                        qT_h[2 * qc + half][:],
                                        start=True, stop=True)
                                ex = exp_pool.tile([128, 1024], BF16,
                                                   tag="ex")
                                nc.scalar.activation(ex[:], lg[:], AF.Exp)
                                exs[st] = ex
                                # denominator: two bf16 chains on the DVE
                                tgt = acc_e if st % 2 == 0 else acc_o
                                if st < 2:
                                    nc.vector.tensor_copy(tgt[:], ex[:])
                                else:
                                    nc.vector.tensor_add(tgt[:], tgt[:], ex[:])
                            if st >= 2:
                                ex = exs.pop(st - 2)
                                sp = st - 2
                                for half in range(2):
                                    exh = ex[:, half * 512:(half + 1) * 512]
                                    nc.tensor.matmul(
                                        wv_ps[:, half * 512:(half + 1) * 512],
                                        vnat[sp][:, h * 128:(h + 1) * 128],
                                        exh,
                                        start=(sp == 0), stop=(sp == ST - 1))
                        # normalize: combine chains, all-reduce+broadcast on
                        # GpSimd, accurate reciprocal on DVE, scale.
                        sumf = rp.tile([128, 1024], F32, tag="sumf")
                        nc.vector.tensor_add(sumf[:], acc_e[:], acc_o[:])
                        bc = rp.tile([128, 1024], F32, tag="bc")
                        nc.gpsimd.partition_all_reduce(
                            bc[:], sumf[:], channels=128,
                            reduce_op=bass_isa.ReduceOp.add)
                        rc = rp.tile([128, 1024], F32, tag="rc")
                        scr = rp.tile([128, 1024], F32, tag="scr")
                        nc.vector.reciprocal_approx_accurate(rc[:], bc[:], scr[:])
                        for half in range(2):
                            nc.vector.tensor_mul(
                                wvn[h][2 * qc + half][:],
                                wv_ps[:, half * 512:(half + 1) * 512],
                                rc[:, half * 512:(half + 1) * 512])

                # ---- phase 3: partial out-projection [s, o] ----
                for qt in range(ST):
                    ob = obp.tile([128, D], BF16, tag="ob")
                    for opair in range(2):
                        op = ps2.tile([128, 1024], F32,
                                      tag=("lg" if opair == 0 else "wv"),
                                      bufs=2, name=f"op{qt}_{opair}")
                        for oci in range(2):
                            oc = opair * 2 + oci
                            for h in range(HPC):
                                nc.tensor.matmul(
                                    op[:, oci * 512:(oci + 1) * 512],
                                    wvn[h][qt // 4][:, (qt % 4) * 128:
                                                    (qt % 4 + 1) * 128],
                                    wo_sb[h][:, oc * 512:(oc + 1) * 512],
                                    start=(h == 0), stop=(h == HPC - 1))
                        for oci in range(2):
                            oc = opair * 2 + oci
                            dst = ob[:, oc * 512:(oc + 1) * 512]
                            src = op[:, oci * 512:(oci + 1) * 512]
                            if oci == 0:
                                nc.vector.tensor_copy(dst, src)
                            else:
                                nc.scalar.copy(dst, src)
                    nc.sync.dma_start(out_d[qt * 128:(qt + 1) * 128, :], ob[:])

    nc.compile()
    return nc


def kernel(x, qkv_weight, qkv_bias, attn_out_weight, attn_out_bias,
           position_ids):
    global _BUILT, _LAST_RESULTS
    x = np.asarray(x, np.float32)
    qkv_weight = np.asarray(qkv_weight, np.float32)
    qkv_bias = np.asarray(qkv_bias, np.float32)
    attn_out_weight = np.asarray(attn_out_weight, np.float32)
    attn_out_bias = np.asarray(attn_out_bias, np.float32)
    position_ids = np.asarray(position_ids)

    half = HD // 2
    xT = np.ascontiguousarray(x[:, 0, :].T)
    inv_freq = 1.0 / (10000.0 ** (np.arange(0, half, 2, dtype=np.float32) / half))
    pos1 = position_ids[0, 0, :].astype(np.float32)
    pos2 = position_ids[0, 1, :].astype(np.float32)
    ang1 = np.concatenate([inv_freq[:, None] * pos1[None, :]] * 2, axis=0)
    ang2 = np.concatenate([inv_freq[:, None] * pos2[None, :]] * 2, axis=0)
    COS = np.ascontiguousarray(
        np.concatenate([np.cos(ang1), np.cos(ang2)], axis=0), dtype=np.float32)
    SIN = np.ascontiguousarray(
        np.concatenate([np.sin(ang1), np.sin(ang2)], axis=0), dtype=np.float32)
    # fold the rotate-half signs into the sin table: the rotated operand is a
    # plain 32-partition swap, with -1 on the first half of each 64-block.
    p = np.arange(128)
    sign = np.where((p % 64) < 32, -1.0, 1.0).astype(np.float32)
    SINS = np.ascontiguousarray(SIN * sign[:, None])
    rotp = np.where((p % 64) < 32, p + 32, p - 32)   # rot[p] = zb[rotp[p]]

    in_maps = []
    for c in range(NCORES):
        c0 = c * HPC * HD                     # first q column of this core
        wq = qkv_weight[:, c0:c0 + HPC * HD]
        wk = qkv_weight[:, D + c0:D + c0 + HPC * HD]
        wv = qkv_weight[:, 2 * D + c0:2 * D + c0 + HPC * HD]
        bq = qkv_bias[c0:c0 + HPC * HD]
        bk = qkv_bias[D + c0:D + c0 + HPC * HD]
        wo = attn_out_weight[c0:c0 + HPC * HD, :]
        wqk = np.ascontiguousarray(np.concatenate([wq, wk], axis=1))
        bqk = np.ascontiguousarray(
            np.stack([bq[:128], bq[128:], bk[:128], bk[128:]], axis=1))
        rbqk = np.ascontiguousarray(bqk[rotp, :])
        in_maps.append({
            "xT": xT,
            "wqk": wqk,
            "bqk": bqk,
            "rbqk": rbqk,
            "wv": np.ascontiguousarray(wv),
            "cos": COS,
            "sin": SINS,
            "wo": np.ascontiguousarray(wo.astype(ml_dtypes.bfloat16)),
        })

    if _BUILT is None:
        _BUILT = _build()
    res = run_bass_kernel_spmd(_BUILT, in_maps, core_ids=list(range(NCORES)))
    _LAST_RESULTS = res

    acc = np.zeros((S, D), dtype=np.float32)
    for r in res.results:
        acc += r["out"].astype(np.float32)
    bv = qkv_bias[2 * D:3 * D]
    acc += (bv @ attn_out_weight)[None, :] + attn_out_bias[None, :]
    return acc.reshape(S, 1, D).astype(np.float32)
